# revision 3
# baseline (speedup 1.0000x reference)
"""minLSTM (2-layer, B=4, S=4096, D=1024) on 8 Trainium2 NeuronCores.

Sharding: core k -> (batch b = k//2, channel half h = k%2).
PE does bf16 matmuls at the 216ns/instruction column-rate floor (measured;
fp8-DR gives 2x FLOPs but 1-pass fp8 fails the accuracy gate and any
compensated variant needs >= as many instructions as bf16, so bf16 is the
PE-optimal dtype here). The elementwise path is restructured so every other
engine stays under the PE's 27.6us/unit:
  - ACT: exactly 4 sigmoids per chunk (bf16 out) + ONE wide [128,2048]
    reciprocal per unit, so only 2 act-table swaps per unit (was 168us of
    table loads in the 6-op/chunk baseline),
  - g = max(cell+bc+0.5, sig(cell+bc)) fused into one DVE STT reading the
    cell psum with a per-partition bias scalar (kills the Identity ACT op),
  - ssum/a-mult on Pool, btn/scan/h on DVE, all bf16 (DVE 2x mode).
x-tile loads are prefetched 3 units ahead on the in-order SP DMA queue;
between layers, channel-half pairs exchange h1 (bf16) via pairwise AllGather.

Self-contained: hardcodes shapes; only imports the system concourse repo.
"""
import sys

if '/opt/trn_rl_repo' not in sys.path:
    sys.path.insert(0, '/opt/trn_rl_repo')

import numpy as np

B, S, D = 4, 4096, 1024
NCORES = 8
HALF = D // 2           # channels per core: 512
NCHUNK = HALF // 128    # 4 partition chunks of 128 channels
NKT = D // 128          # 8 contraction k-tiles
TBLK = 512              # token block
NBLK = S // TBLK        # 8 token blocks
GCH = 4 * HALF          # gate channels per core: 2048
PF = 3                  # x-tile prefetch depth in (layer, block) units

_CACHE = {}


def _split_multi_waits(nc):
    """This walrus build rejects >1 sync wait per instruction. Hoist extra
    waits onto same-engine NoOps inserted just before; engine-queue program
    order makes this semantically identical."""
    from concourse import mybir
    n = 0
    for fn in nc.m.functions:
        for blk in fn.blocks:
            insts = list(blk.instructions)
            new = []
            changed = False
            for inst in insts:
                si = inst.sync_info
                ow = list(si.on_wait) if si is not None and si.on_wait else []
                if len(ow) > 1:
                    changed = True
                    for w in ow[:-1]:
                        n += 1
                        nop = mybir.InstNoOp(name=f"I-wsplit-{n}", ins=[], outs=[])
                        nop.engine = inst.engine
                        nop.sync_info = mybir.SyncInfo(on_wait=[w], on_update=[])
                        new.append(nop)
                    si.on_wait = [ow[-1]]
                new.append(inst)
            if changed:
                blk.instructions = new
    return n


def _build_nc():
    import concourse.bass as bass
    import concourse.mybir as mybir
    import concourse.tile as tile

    f32 = mybir.dt.float32
    bf16 = mybir.dt.bfloat16
    AF = mybir.ActivationFunctionType
    ALU = mybir.AluOpType

    nc = bass.Bass("TRN2", target_bir_lowering=False, debug=False,
                   num_devices=NCORES)

    # xT is shipped host-side as the SBUF image [128, t*4096 + k*512 + c]
    # so each layer-0 unit loads with one contiguous DMA.
    xT_d = nc.dram_tensor("xT", [128, NBLK * NKT * 512], bf16,
                          kind="ExternalInput").ap()
    # w0t is shipped host-side in the exact SBUF image layout
    # [128, j*4096 + k*512 + c] so each chunk-quarter load is one fully
    # contiguous DMA (the strided rearrange path runs ~3x slower).
    w0_d = nc.dram_tensor("w0t", [128, 4 * NKT * 512], bf16,
                          kind="ExternalInput").ap()
    w1_d = nc.dram_tensor("w1t", [D, GCH], bf16, kind="ExternalInput").ap()
    ba_d = [nc.dram_tensor(f"b{l}a", [128, 16], f32, kind="ExternalInput").ap()
            for l in range(2)]
    bc_d = [nc.dram_tensor(f"b{l}c", [128, 4], f32, kind="ExternalInput").ap()
            for l in range(2)]
    cp_d = [nc.dram_tensor(f"cp{l}", [128, 4], f32, kind="ExternalInput").ap()
            for l in range(2)]
    h2t_d = nc.dram_tensor("h2t", [HALF, S], bf16, kind="ExternalOutput").ap()

    with tile.TileContext(nc) as tc:
        with tc.tile_pool(name="wp", bufs=1) as wp, \
             tc.tile_pool(name="xkp", bufs=PF + 1) as xkp, \
             tc.tile_pool(name="gp", bufs=2) as gp, \
             tc.tile_pool(name="cgp", bufs=3) as cgp, \
             tc.tile_pool(name="cp", bufs=1) as cpool, \
             tc.tile_pool(name="psum", bufs=8, space="PSUM") as psum, \
             tc.tile_pool(name="dstage", bufs=4, space="DRAM") as dstage, \
             tc.tile_pool(name="dfull", bufs=8, space="DRAM") as dfull:

            # h1 gathered blocks must persist through layer 2: 8 live tiles
            h1f = [dfull.tile([D, TBLK], bf16, tag="h1f", name=f"h1f{t}")
                   for t in range(NBLK)]

            # Weight layout (host side): gate-channel index ct = j*4 + q so a
            # chunk j's four gate slices are one contiguous [128,512] span.
            # Layer-0 weights stream per k on the SP queue right behind the
            # first x block; layer-1 weights go on the ACT hwdge queue so
            # they never delay layer-0's pipeline.
            ba = {}
            bc = {}
            cp = {}
            for l in range(2):
                ba[l] = cpool.tile([128, 16], f32, tag=f"ba{l}", name=f"ba{l}")
                bc[l] = cpool.tile([128, 4], f32, tag=f"bc{l}", name=f"bc{l}")
                cp[l] = cpool.tile([128, 4], f32, tag=f"cp{l}", name=f"cp{l}")
            # Layer-0 weights as one [128, 8k, 512] tile per chunk-quarter,
            # each filled by a single strided DMA, so the first chunk's
            # k-outer matmuls start ~1.5us in instead of stalling ~14us
            # behind whole-k [128,2048] transfers. A chunk j's four gate
            # columns are exactly quarter j of the ct-ordered weight
            # matrix. Layer-1 keeps whole-k tiles.
            w0q = [wp.tile([128, NKT * 512], bf16, tag=f"W0q{j}",
                           name=f"w0q{j}") for j in range(NCHUNK)]
            w1_ks = [wp.tile([128, GCH], bf16, tag=f"Wk1_{k}",
                             name=f"w1_{k}") for k in range(NKT)]

            def w_lhsT(l, k, ct):
                if l == 0:
                    off = k * 512 + (ct % 4) * 128
                    return w0q[ct // 4][:, off:off + 128]
                return w1_ks[k][:, ct * 128:(ct + 1) * 128]

            units = [(l, t) for l in range(2) for t in range(NBLK)]
            xk_tiles = {}

            def load_unit(u):
                # per-k tiles on purpose: 8 small DMAs interleave with the
                # h-store traffic on the in-order SP queue, where a single
                # 1MB transfer head-blocks it for ~4us (measured +15us of
                # PE gaps with the single-DMA variant)
                l, t = units[u]
                xk_ks = []
                for k in range(NKT):
                    xkt = xkp.tile([128, TBLK], bf16, tag=f"xk{k}",
                                   name=f"xk{l}_{t}_{k}")
                    if l == 0:
                        src = xT_d[:, t * 4096 + k * 512:
                                   t * 4096 + (k + 1) * 512]
                    else:
                        src = h1f[t][k * 128:(k + 1) * 128, :]
                    nc.sync.dma_start(xkt[:], src)
                    xk_ks.append(xkt)
                xk_tiles[u] = xk_ks

            def x_rhs(l, xk_ks, k):
                return xk_ks[k][:]

            # Startup order on the SP queue: quarter-0 weights (two halves,
            # so the first matmul's dependency lands after ~0.7us), unit-0
            # x tiles, then quarters 1..3 (chunk j's k-inner matmuls begin
            # at ~7us*j, after quarter j lands). Biases and prefetched
            # units follow. Layer-1 weights ride the ACT hwdge queue so
            # they never delay layer-0's pipeline.
            # Startup on the SP queue in the measured-best order (chunk-0
            # weights in two pieces interleaved with unit-0's x tiles); the
            # only change vs the proven config is that x comes from the
            # host-packed image, so each [128,512] load is one contiguous
            # 4KB-per-partition burst instead of 128 strided 1KB rows.
            xk0 = []

            def xk0_load(k):
                xkt = xkp.tile([128, TBLK], bf16, tag=f"xk{k}",
                               name=f"xk0_0_{k}")
                nc.sync.dma_start(
                    xkt[:], xT_d[:, k * 512:(k + 1) * 512])
                xk0.append(xkt)

            nc.sync.dma_start(w0q[0][:, 0:2048], w0_d[:, 0:2048])
            xk0_load(0)
            xk0_load(1)
            nc.sync.dma_start(w0q[0][:, 2048:4096], w0_d[:, 2048:4096])
            for k in range(2, NKT):
                xk0_load(k)
            xk_tiles[0] = xk0
            for j in range(1, NCHUNK):
                nc.sync.dma_start(w0q[j][:],
                                  w0_d[:, j * 4096:(j + 1) * 4096])
            for l in range(2):
                nc.sync.dma_start(ba[l][:], ba_d[l][:])
                nc.sync.dma_start(bc[l][:], bc_d[l][:])
                nc.sync.dma_start(cp[l][:], cp_d[l][:])
            for u in range(1, min(PF, len(units))):
                load_unit(u)
            for k in range(NKT):
                nc.scalar.dma_start(w1_ks[k][:],
                                    w1_d[k * 128:(k + 1) * 128, :])

            def act_recip(out_ap, in_ap):
                """ACT-table reciprocal (bass blocks AF.Reciprocal in
                activation(); measured 1.2e-5 max rel err on [9e-5, 2],
                far inside this kernel's tolerance)."""
                eng = nc.scalar
                ins_ = [eng.lower_ap(in_ap)]
                for argv in (0.0, 1.0, 0.0):  # bias, scale, alpha imms
                    ins_.append(mybir.ImmediateValue(dtype=f32, value=argv))
                eng.add_instruction(mybir.InstActivation(
                    name=nc.get_next_instruction_name(),
                    func=AF.Reciprocal,
                    ins=ins_,
                    outs=[eng.lower_ap(out_ap)]))

            carry = {0: [None] * NCHUNK, 1: [None] * NCHUNK}

            def emit_phase2(l, t, j, tiles, r_w, cp, carry, h1own):
                sf, si, so, g = tiles[j]
                a = gp.tile([128, TBLK], bf16, tag="a", bufs=3,
                            name=f"a{l}{t}{j}")
                nc.gpsimd.tensor_tensor(
                    a[:], sf[:], r_w[:, j * TBLK:(j + 1) * TBLK],
                    ALU.mult)
                btn = gp.tile([128, TBLK], bf16, tag="btn", bufs=2,
                              name=f"bt{l}{t}{j}")
                nc.vector.scalar_tensor_tensor(
                    btn[:], a[:], 1.0, g[:], ALU.subtract, ALU.mult)
                c = cgp.tile([128, TBLK], bf16, tag=f"c{j}",
                             name=f"c{l}{t}{j}")
                init = cp[l][:, j:j + 1] if t == 0 else carry[l][j]
                nc.vector.tensor_tensor_scan(c[:], a[:], btn[:],
                                             init, ALU.mult,
                                             ALU.subtract)
                carry[l][j] = c[:, TBLK - 1:TBLK]
                h = gp.tile([128, TBLK], bf16, tag=f"h{l}", bufs=3,
                            name=f"h{l}{t}{j}")
                nc.vector.tensor_tensor(h[:], so[:], c[:], ALU.mult)
                if l == 0:
                    nc.sync.dma_start(
                        h1own[j * 128:(j + 1) * 128, :], h[:])
                else:
                    nc.sync.dma_start(
                        h2t_d[j * 128:(j + 1) * 128,
                              t * TBLK:(t + 1) * TBLK], h[:])

            for u, (l, t) in enumerate(units):
                if u + PF < len(units):
                    load_unit(u + PF)
                xk_ks = xk_tiles.pop(u)

                if l == 0:
                    h1own = dstage.tile([HALF, TBLK], bf16, tag="h1own",
                                        name=f"h1own{t}")

                def col(j, qi_):
                    return ba[l][:, j * 4 + qi_:j * 4 + qi_ + 1]

                # Phase 1 per chunk: matmuls, the 4 sigmoids (bf16 out), the
                # fused g = max(cell+bc+.5, sg) STT, and the Pool ssum. The
                # psum tiles are fully consumed here, so 8 banks keep two
                # chunks in flight under the PE.
                ssum_w = gp.tile([128, 2048], bf16, tag="ssw",
                                 name=f"ssw{l}{t}")
                r_w = gp.tile([128, 2048], bf16, tag="rw", name=f"rw{l}{t}")
                tiles = {}
                last_unit = (u == len(units) - 1)
                for j in range(NCHUNK):
                    ps = {}
                    for q in ("i", "f", "o", "cell"):
                        ps[q] = psum.tile([128, TBLK], f32, tag="ps",
                                          name=f"ps_{q}{l}_{t}_{j}")
                    # k-inner (8 MMs per PSUM group) avoids the HAM
                    # psum-cycling throttle; the very first chunk goes
                    # k-outer so the PE starts while weight k-tiles are
                    # still arriving (HAM is cold then anyway).
                    if u == 0 and j == 0:
                        mm_order = [(k, qi) for k in range(NKT)
                                    for qi in range(4)]
                    else:
                        mm_order = [(k, qi) for qi in range(4)
                                    for k in range(NKT)]
                    qnames = ("i", "f", "o", "cell")
                    for k, qi in mm_order:
                        ct = j * 4 + qi
                        nc.tensor.matmul(
                            ps[qnames[qi]][:],
                            w_lhsT(l, k, ct),
                            x_rhs(l, xk_ks, k),
                            start=(k == 0), stop=(k == NKT - 1))

                    sf = gp.tile([128, TBLK], bf16, tag="sf", bufs=6,
                                 name=f"sf{l}{t}{j}")
                    nc.scalar.activation(sf[:], ps["f"][:], AF.Sigmoid,
                                         bias=col(j, 1))
                    si = gp.tile([128, TBLK], bf16, tag="si", bufs=6,
                                 name=f"si{l}{t}{j}")
                    nc.scalar.activation(si[:], ps["i"][:], AF.Sigmoid,
                                         bias=col(j, 0))
                    so = gp.tile([128, TBLK], bf16, tag="so", bufs=6,
                                 name=f"so{l}{t}{j}")
                    nc.scalar.activation(so[:], ps["o"][:], AF.Sigmoid,
                                         bias=col(j, 2))
                    sg = gp.tile([128, TBLK], bf16, tag="sg", bufs=2,
                                 name=f"sg{l}{t}{j}")
                    nc.scalar.activation(sg[:], ps["cell"][:], AF.Sigmoid,
                                         bias=col(j, 3))
                    # g = max(cell + bc + 0.5, sigmoid(cell + bc)): one STT
                    # reading the raw cell psum with the (+bc+0.5) column.
                    g = gp.tile([128, TBLK], bf16, tag="g", bufs=6,
                                name=f"g{l}{t}{j}")
                    nc.vector.scalar_tensor_tensor(
                        g[:], ps["cell"][:], bc[l][:, j:j + 1], sg[:],
                        ALU.add, ALU.max)
                    nc.gpsimd.tensor_tensor(
                        ssum_w[:, j * TBLK:(j + 1) * TBLK], sf[:], si[:],
                        ALU.add)
                    tiles[j] = (sf, si, so, g)
                    if last_unit and j == 1:
                        # Tail trim: split the final unit's reciprocal so
                        # chunks 0-1 drain while 2-3 are still on the PE
                        # (costs 2 extra table swaps, once).
                        act_recip(r_w[:, 0:1024], ssum_w[:, 0:1024])
                        for jj in range(2):
                            emit_phase2(l, t, jj, tiles, r_w, cp, carry,
                                        h1own if l == 0 else None)

                if last_unit:
                    act_recip(r_w[:, 1024:2048], ssum_w[:, 1024:2048])
                    phase2_chunks = range(2, NCHUNK)
                else:
                    # ONE wide reciprocal per unit: 2 table swaps per unit.
                    act_recip(r_w[:], ssum_w[:])
                    phase2_chunks = range(NCHUNK)

                # Phase 2 per chunk: normalize, scan, h, store.
                for j in phase2_chunks:
                    emit_phase2(l, t, j, tiles, r_w, cp, carry,
                                h1own if l == 0 else None)

                if l == 0:
                    nc.gpsimd.collective_compute(
                        "AllGather", ALU.bypass,
                        replica_groups=[[0, 1], [2, 3], [4, 5], [6, 7]],
                        ins=[h1own.opt()],
                        outs=[h1f[t].opt()],
                    )

    _split_multi_waits(nc)
    return nc


def _shard_inputs(x, W0, b0, W1, b1, c0_prev, c1_prev):
    import ml_dtypes
    bfdt = ml_dtypes.bfloat16
    x = np.asarray(x, dtype=np.float32)
    in_maps = []
    # SBUF image: [p, t*4096 + k*512 + c] = x[b].T[k*128+p, t*512+c]
    xT = [np.ascontiguousarray(
        x[b].T.astype(bfdt).reshape(NKT, 128, NBLK, TBLK)
        .transpose(1, 2, 0, 3).reshape(128, NBLK * NKT * TBLK))
        for b in range(B)]
    per_layer = []
    for (W, bb) in ((W0, b0), (W1, b1)):
        W = np.asarray(W, dtype=np.float32)
        bb = np.asarray(bb, dtype=np.float32)
        halves = []
        for h in range(2):
            # gate-channel order ct = j*4 + q (chunk-major) so each chunk's
            # four gate weight slices are one contiguous [*, 512] span
            rows = np.concatenate(
                [q * D + h * HALF + j * 128 + np.arange(128)
                 for j in range(4) for q in range(4)])
            wt = np.ascontiguousarray(W[rows, :].T.astype(bfdt))  # (D, GCH)
            ba = np.ascontiguousarray(bb[rows].reshape(16, 128).T)  # (128,16)
            bc = np.ascontiguousarray(
                ba[:, 3::4] + np.float32(0.5))  # cell cols (ct=j*4+3) + 0.5
            halves.append((wt, ba, bc))
        per_layer.append(halves)
    cps = []
    for cprev in (c0_prev, c1_prev):
        cprev = np.asarray(cprev, dtype=np.float32)
        halves = []
        for b in range(B):
            row = []
            for h in range(2):
                seg = cprev[b, 0, h * HALF:(h + 1) * HALF]
                row.append(np.ascontiguousarray(seg.reshape(4, 128).T))
            halves.append(row)
        cps.append(halves)
    for k in range(NCORES):
        b, h = k // 2, k % 2
        m = {"xT": xT[b]}
        for l in range(2):
            wt, ba, bc = per_layer[l][h]
            if l == 0:
                # SBUF image layout: [p, j*4096 + kk*512 + c] so each
                # chunk-quarter is one contiguous DMA on device.
                m["w0t"] = np.ascontiguousarray(
                    wt.reshape(8, 128, 4, 512).transpose(1, 2, 0, 3)
                    .reshape(128, 4 * 8 * 512))
            else:
                m["w1t"] = wt
            m[f"b{l}a"] = ba
            m[f"b{l}c"] = bc
            m[f"cp{l}"] = cps[l][b][h]
        in_maps.append(m)
    return in_maps


def _get_nc():
    if "nc" not in _CACHE:
        _CACHE["nc"] = _build_nc()
    return _CACHE["nc"]


def kernel(x, W0, b0, W1, b1, c0_prev, c1_prev):
    from concourse.bass_utils import run_bass_kernel_spmd

    nc = _get_nc()
    in_maps = _shard_inputs(x, W0, b0, W1, b1, c0_prev, c1_prev)
    res = run_bass_kernel_spmd(nc, in_maps, list(range(NCORES)))
    out = np.empty((B, S, D), dtype=np.float32)
    for k in range(NCORES):
        b, h = k // 2, k % 2
        out[b, :, h * HALF:(h + 1) * HALF] = \
            res.results[k]["h2t"].astype(np.float32).T
    return out
